# revision 1
# baseline (speedup 1.0000x reference)
"""BatchAllTripletLoss Trainium2 kernel.

Problem (hardcoded): x (64, 256, 256) f32, y (64, 256) int64 with
y[p, i] = i // 8 (32 classes x 8 members, uniform, identical across parts).
Output: per-part batch-all triplet loss, shape (64,) f32.

Math per part:
  D[i,j]  = euclidean distance matrix from x_p
  pos(i)  = 8 same-class columns (incl. self), neg(i) = 248 others
  loss_p  = mean over nonzero of relu(margin + D[i,j] - D[i,l]),
            j in pos(i), l in neg(i)

Device strategy (8 NeuronCores, 8 parts each, fully independent):
  - gram via PE (bf16), squared-norm row/col folded in via rank-1 matmuls
  - +L on same-class columns (rank-32 class-indicator matmul) so the
    "pos" columns vanish from both the relu-sum and the count (no
    correction pass needed); a second accumulation flips the pollution
    to extract the true pos distances via a strided max-reduce.
  - ACT sqrt (scale=-2, bias=sq_col) -> D' bf16, accum -> row sums
  - epilogue per pos-slot t: relu-sum via   sum_l max(P'+m, D') - sum_l D'
    and count via sum_l 1[D' < P'+m]; split across DVE (tensor_scalar
    + accum) and ACT (Relu/Sign with per-partition bias + accum).
  - per-core output: (S_p, N_p) pairs; host does the final division.
"""

import os
import numpy as np
from contextlib import ExitStack

import concourse.bass as bass
import concourse.bacc as bacc_mod
import concourse.mybir as mybir
import concourse.tile as tile

F32 = mybir.dt.float32
BF16 = mybir.dt.bfloat16
ALU = mybir.AluOpType
ACTF = mybir.ActivationFunctionType

# problem constants
P_TOT, N, C = 64, 256, 256
K, NCLS = 8, 32
MARGIN = 0.2
NCORES = 8
PPC = P_TOT // NCORES  # parts per core
HALVES = 2  # anchor halves of 128
LBIG = float(2 << 19)  # 2^20 pollution offset
EPS = 4.0  # diagonal-only safety offset (eps * I via PE)
N_DVE_T = 6  # pos-slots handled on DVE; rest on ACT
ACC_W = 17  # accumulator tile cols: 0-7 sum-ish, 8-15 cnt-ish, 16 rowD


def build_kernel(do_compile=True, reps=1):
    nc = bacc_mod.Bacc()
    x_in = nc.declare_dram_parameter("x", [PPC * N, C], F32, isOutput=False)
    sn_out = nc.declare_dram_parameter("sn", [1, 2 * PPC], F32, isOutput=True)

    with tile.TileContext(nc) as tc, ExitStack() as ctx:
        consts = ctx.enter_context(tc.tile_pool(name="consts", bufs=1))
        xpool = ctx.enter_context(tc.tile_pool(name="xpool", bufs=2))
        xtpool = ctx.enter_context(tc.tile_pool(name="xtpool", bufs=2))
        dpool = ctx.enter_context(tc.tile_pool(name="dpool", bufs=2))
        small = ctx.enter_context(tc.tile_pool(name="small", bufs=3))
        trash = ctx.enter_context(tc.tile_pool(name="trash", bufs=4))
        accp = ctx.enter_context(tc.tile_pool(name="accp", bufs=2))
        psum = ctx.enter_context(tc.tile_pool(name="psum", bufs=3, space="PSUM"))
        psmall = ctx.enter_context(tc.tile_pool(name="psmall", bufs=1, space="PSUM"))

        # ---- one-time constants ----
        # class-indicator CT[k, i] = 1[i // 8 == k]  (32 partitions x 256)
        ct_one = consts.tile([NCLS, N], BF16, tag="ct1")
        nc.vector.memset(ct_one[:], 1.0)
        nc.gpsimd.affine_select(
            ct_one[:], ct_one[:], pattern=[[1, NCLS], [0, K]],
            compare_op=ALU.is_equal, fill=0.0, base=0, channel_multiplier=-1,
        )
        ct_a = consts.tile([NCLS, N], BF16, tag="cta")  # -L/2 * CT
        nc.vector.memset(ct_a[:], -LBIG / 2)
        nc.gpsimd.affine_select(
            ct_a[:], ct_a[:], pattern=[[1, NCLS], [0, K]],
            compare_op=ALU.is_equal, fill=0.0, base=0, channel_multiplier=-1,
        )
        ct_b = consts.tile([NCLS, N], BF16, tag="ctb")  # +L * CT
        nc.vector.memset(ct_b[:], LBIG)
        nc.gpsimd.affine_select(
            ct_b[:], ct_b[:], pattern=[[1, NCLS], [0, K]],
            compare_op=ALU.is_equal, fill=0.0, base=0, channel_multiplier=-1,
        )
        ct_half = consts.tile([NCLS, N], BF16, tag="cth")  # CT - 0.5
        nc.vector.memset(ct_half[:], 0.5)
        nc.gpsimd.affine_select(
            ct_half[:], ct_half[:], pattern=[[1, NCLS], [0, K]],
            compare_op=ALU.is_equal, fill=-0.5, base=0, channel_multiplier=-1,
        )
        ident = consts.tile([128, 128], BF16, tag="ident")
        nc.vector.memset(ident[:], 1.0)
        nc.gpsimd.affine_select(
            ident[:], ident[:], pattern=[[1, 128]],
            compare_op=ALU.is_equal, fill=0.0, base=0, channel_multiplier=-1,
        )
        neghalf = consts.tile([1, 128], BF16, tag="neghalf")  # mm3 lhsT
        nc.vector.memset(neghalf[:], -0.5)
        ieps = consts.tile([128, 128], BF16, tag="ieps")  # -EPS/2 * I
        nc.vector.memset(ieps[:], -EPS / 2)
        nc.gpsimd.affine_select(
            ieps[:], ieps[:], pattern=[[1, 128]],
            compare_op=ALU.is_equal, fill=0.0, base=0, channel_multiplier=-1,
        )
        ishift = []
        for h in range(HALVES):
            t_ish = consts.tile([128, N], BF16, tag=f"ish{h}", name=f"ish{h}")
            nc.vector.memset(t_ish[:], 1.0)
            nc.gpsimd.affine_select(
                t_ish[:], t_ish[:], pattern=[[1, N]],
                compare_op=ALU.is_equal, fill=0.0, base=-128 * h,
                channel_multiplier=-1,
            )
            ishift.append(t_ish)
        ones_col = consts.tile([128, 1], F32, tag="ones_col")  # final col-sum lhsT
        nc.vector.memset(ones_col[:], 1.0)

        # persistent cross-part tiles
        fin_ps = psmall.tile([1, ACC_W * PPC * HALVES], F32, tag="fin_ps")

        for p in [pp for _ in range(reps) for pp in range(PPC)]:
            # ---- load x rows (f32) and transposed (f32 via 4-byte xbar) ----
            xf = [xpool.tile([128, C], F32, tag="xf", name="xf", bufs=16) for _ in range(HALVES)]
            for h in range(HALVES):
                nc.sync.dma_start(xf[h][:], x_in[p * N + 128 * h: p * N + 128 * (h + 1), :])
            # cast rows to bf16 (DVE), then xbar-transpose
            xb = [xpool.tile([128, C], BF16, tag="xb", name="xb") for _ in range(HALVES)]
            for h in range(HALVES):
                nc.vector.tensor_copy(xb[h][:], xf[h][:])
            # transpose via PE into PSUM, one copy back to SBUF
            xtps = psum.tile([128, 2 * N], BF16, tag="xtps", name="xtps", bufs=2)
            for cchunk in range(2):
                for h in range(HALVES):
                    nc.tensor.transpose(
                        xtps[:, 256 * cchunk + 128 * h: 256 * cchunk + 128 * (h + 1)],
                        xb[h][:, 128 * cchunk: 128 * (cchunk + 1)],
                        ident[:],
                    )
            xtb_all = xtpool.tile([128, 2 * N], BF16, tag="xtb", name="xtb")
            nc.vector.tensor_copy(xtb_all[:], xtps[:])
            xtb = [xtb_all[:, 0:N], xtb_all[:, N: 2 * N]]

            # ---- squared norms (per-anchor) ----
            sqcol = []
            sqcol_b = []
            for h in range(HALVES):
                sc = small.tile([128, 1], F32, tag="sqcol")
                st = trash.tile([128, C], BF16, tag="trash_sq")
                nc.scalar.activation(st[:], xb[h][:], ACTF.Square, accum_out=sc[:])
                sqcol.append(sc)
                scb = small.tile([128, 1], BF16, tag="sqcolb")
                nc.vector.tensor_copy(scb[:], sc[:])
                sqcol_b.append(scb)
            # sq as a row: PE-transpose both halves into a [1, N] psum strip
            sqrow_ps = psum.tile([1, N], BF16, tag="sqrow_ps", name="sqrow_ps", bufs=2)
            for h in range(HALVES):
                nc.tensor.transpose(
                    sqrow_ps[0:1, 128 * h: 128 * (h + 1)], sqcol_b[h][:], ident[:]
                )
            sqrow = small.tile([1, N], BF16, tag="sqrow")
            nc.scalar.activation(
                sqrow[:], sqrow_ps[:], ACTF.Copy, bias=0.0, scale=1.0
            )

            # ---- per half: psum1 = gram - (sq_row+eps)/2 - L/2*B ; D' = sqrt ----
            acc = [accp.tile([128, ACC_W], F32, tag="acc", name="acc") for _ in range(HALVES)]
            dmat = []
            pm = []
            for h in range(HALVES):
                ps = psum.tile([128, N], F32, tag="ps")
                nc.tensor.matmul(
                    ps[:], xtb[0][:, 128 * h: 128 * (h + 1)], xtb[0][:],
                    start=True, stop=False,
                )
                nc.tensor.matmul(
                    ps[:], xtb[1][:, 128 * h: 128 * (h + 1)], xtb[1][:],
                    start=False, stop=False,
                )
                nc.tensor.matmul(
                    ps[:], neghalf[:, 0:128], sqrow[:], start=False, stop=False,
                )
                nc.tensor.matmul(
                    ps[:], ct_a[:, 128 * h: 128 * (h + 1)], ct_one[:],
                    start=False, stop=True,
                )
                dm = dpool.tile([128, N], F32, tag="dmat")
                nc.scalar.activation(
                    dm[:], ps[:], ACTF.Sqrt, bias=sqcol[h][:], scale=-2.0,
                    accum_out=acc[h][:, 16:17],
                )
                dmat.append(dm)

                # flip pollution: psum2 = psum1 + L*(B - 1/2)  -> s2 = s + L(1-B)
                nc.tensor.matmul(
                    ps[:], ct_b[:, 128 * h: 128 * (h + 1)], ct_half[:],
                    start=False, stop=False, skip_group_check=True,
                )
                nc.tensor.matmul(
                    ps[:], ieps[:], ishift[h][:],
                    start=False, stop=True, skip_group_check=True,
                )
                # true pos squared-dists: max over class-blocks of raw psum
                spos = small.tile([128, K], F32, tag="spos")
                nc.vector.tensor_reduce(
                    spos[:],
                    ps[:].rearrange("p (h t) -> p t h", h=NCLS, t=K),
                    axis=mybir.AxisListType.X, op=ALU.max,
                )
                pp = small.tile([128, K], F32, tag="pp")
                nc.scalar.activation(pp[:], spos[:], ACTF.Sqrt, bias=sqcol[h][:], scale=-2.0)
                pmh = small.tile([128, K], F32, tag="pmh")
                nc.vector.tensor_scalar(pmh[:], pp[:], MARGIN, None, op0=ALU.add)
                pm.append(pmh)

            # ---- epilogue: per pos-slot t ----
            for h in range(HALVES):
                for t in range(K):
                    if t < N_DVE_T:
                        t1 = trash.tile([128, N], BF16, tag="trA")
                        nc.vector.tensor_scalar(
                            t1[:], dmat[h][:], pm[h][:, t: t + 1], None,
                            op0=ALU.max, op1=ALU.add, accum_out=acc[h][:, t: t + 1],
                        )
                        t2 = trash.tile([128, N], BF16, tag="trB")
                        nc.vector.tensor_scalar(
                            t2[:], dmat[h][:], pm[h][:, t: t + 1], None,
                            op0=ALU.is_lt, op1=ALU.add, accum_out=acc[h][:, 8 + t: 9 + t],
                        )
                    else:
                        t1 = trash.tile([128, N], BF16, tag="trC")
                        nc.scalar.activation(
                            t1[:], dmat[h][:], ACTF.Relu,
                            bias=pm[h][:, t: t + 1], scale=-1.0,
                            accum_out=acc[h][:, t: t + 1],
                        )
                        t2 = trash.tile([128, N], BF16, tag="trD")
                        nc.scalar.activation(
                            t2[:], dmat[h][:], ACTF.Sign,
                            bias=pm[h][:, t: t + 1], scale=-1.0,
                            accum_out=acc[h][:, 8 + t: 9 + t],
                        )

            # ---- cross-partition col sums via PE ----
            for h in range(HALVES):
                j = p * HALVES + h
                nc.tensor.matmul(
                    fin_ps[0:1, ACC_W * j: ACC_W * (j + 1)], ones_col[:], acc[h][:],
                    start=True, stop=True,
                )

        # ---- finalize: [1, 20*16] -> (S_p, N_p) x 8 ----
        fin = small.tile([1, ACC_W * PPC * HALVES], F32, tag="fin")
        nc.vector.tensor_copy(fin[:], fin_ps[:])
        # sum-ish part: sum_t cols 0..7 per (p,h); DVE slots hold max-sums
        # (need - 256*N_DVE... no: minus N_DVE_T * rowD), ACT slots hold
        # relu-sums directly.
        ssum = small.tile([1, PPC * HALVES], F32, tag="ssum")
        nc.vector.tensor_reduce(
            ssum[:],
            fin[:].rearrange("o (j c) -> o j c", j=PPC * HALVES, c=ACC_W)[:, :, 0:8],
            axis=mybir.AxisListType.X, op=ALU.add,
        )
        rowd = small.tile([1, PPC * HALVES], F32, tag="rowd")
        nc.vector.tensor_copy(
            rowd[:],
            fin[:].rearrange("o (j c) -> o j c", j=PPC * HALVES, c=ACC_W)[:, :, 16:17],
        )
        nc.vector.tensor_scalar(rowd[:], rowd[:], float(N_DVE_T), None, op0=ALU.mult)
        nc.vector.tensor_tensor(ssum[:], ssum[:], rowd[:], op=ALU.subtract)
        # count part: cols 8..8+NDVE-1 are counts; cols 8+NDVE..15 are sign sums
        cnta = small.tile([1, PPC * HALVES], F32, tag="cnta")
        nc.vector.tensor_reduce(
            cnta[:],
            fin[:].rearrange("o (j c) -> o j c", j=PPC * HALVES, c=ACC_W)[:, :, 8: 8 + N_DVE_T],
            axis=mybir.AxisListType.X, op=ALU.add,
        )
        nacts = K - N_DVE_T
        if nacts > 0:
            cntb = small.tile([1, PPC * HALVES], F32, tag="cntb")
            nc.vector.tensor_reduce(
                cntb[:],
                fin[:].rearrange("o (j c) -> o j c", j=PPC * HALVES, c=ACC_W)[:, :, 8 + N_DVE_T: 16],
                axis=mybir.AxisListType.X, op=ALU.add,
            )
            # counts += (sign_sum + 256*nacts)/2 ; per half partition-sum is
            # over 128 partitions so the "+256" per (anchor,t) becomes
            # 128*256*nacts per (p,h) column after the PE col-sum.
            nc.vector.tensor_scalar(
                cntb[:], cntb[:], 0.5, float(128 * N * nacts / 2), op0=ALU.mult, op1=ALU.add
            )
            nc.vector.tensor_tensor(cnta[:], cnta[:], cntb[:], op=ALU.add)
        # pair the two halves: [1, 8, 2] -> [1, 8]
        s_p = small.tile([1, PPC], F32, tag="s_p")
        nc.vector.tensor_reduce(
            s_p[:], ssum[:].rearrange("o (p h) -> o p h", p=PPC, h=HALVES),
            axis=mybir.AxisListType.X, op=ALU.add,
        )
        n_p = small.tile([1, PPC], F32, tag="n_p")
        nc.vector.tensor_reduce(
            n_p[:], cnta[:].rearrange("o (p h) -> o p h", p=PPC, h=HALVES),
            axis=mybir.AxisListType.X, op=ALU.add,
        )
        both = small.tile([1, 2 * PPC], F32, tag="both")
        nc.vector.tensor_copy(both[:, 0:PPC], s_p[:])
        nc.vector.tensor_copy(both[:, PPC: 2 * PPC], n_p[:])
        nc.sync.dma_start(sn_out[:], both[:])

    if do_compile:
        nc.compile()
    return nc


_NC_CACHE = None


def _get_nc():
    global _NC_CACHE
    if _NC_CACHE is None:
        _NC_CACHE = build_kernel()
    return _NC_CACHE


def kernel(x: np.ndarray, y: np.ndarray) -> np.ndarray:
    from concourse.bass_utils import run_bass_kernel_spmd

    x = np.asarray(x)
    y = np.asarray(y)
    assert x.shape == (P_TOT, N, C) and y.shape == (P_TOT, N)
    # the kernel hardcodes the class structure; verify it holds
    expect = np.repeat(np.arange(NCLS, dtype=np.int64), K)
    assert np.array_equal(y, np.broadcast_to(expect, (P_TOT, N))), (
        "kernel requires y[p, i] == i // 8"
    )
    nc = _get_nc()
    xs = np.ascontiguousarray(x.reshape(NCORES, PPC * N, C).astype(np.float32))
    in_maps = [{"x": xs[i]} for i in range(NCORES)]
    res = run_bass_kernel_spmd(nc, in_maps, list(range(NCORES)))
    out = np.empty((P_TOT,), np.float32)
    for i in range(NCORES):
        sn = res.results[i]["sn"].reshape(2 * PPC)
        s, n = sn[:PPC], sn[PPC:]
        out[i * PPC: (i + 1) * PPC] = np.where(n <= 0, 0.0, s / np.maximum(n, 1.0))
    return out



# revision 6
# speedup vs baseline: 1.0576x; 1.0576x over previous
"""BatchAllTripletLoss Trainium2 kernel (v2).

Problem (hardcoded): x (64, 256, 256) f32, y (64, 256) int64 with
y[p, i] = i // 8 (32 classes x 8 members, uniform, identical across parts).
Output: per-part batch-all triplet loss, shape (64,) f32.

Math per part:
  D[i,j]  = euclidean distance matrix from x_p
  pos(i)  = 8 same-class columns (incl. self), neg(i) = 248 others
  loss_p  = mean over nonzero of relu(margin + D[i,j] - D[i,l]),
            j in pos(i), l in neg(i)

v2 design (vs v1 baseline):
  - dmat kept in BF16 so the DVE epilogue ops qualify for the 2x/4x
    fast modes (all non-scalar operands 2-byte packed SBUF).
  - relu-sums computed directly per pos-slot via scalar_tensor_tensor:
    (D max pm_t) - D = relu(pm_t - D), accumulated in-op. No rowD
    correction pass, no Sign/N-correction bookkeeping.
  - pollution-flip PSUM group removed entirely: same-class entries sit
    at s+L in the single accumulation; a strided min-reduce over the
    16-class block of the anchor's half + ACT sqrt with bias sqcol-L
    recovers the true pos distances. (Diagonal NaN guard: clamp
    spos <= (sqcol-L)/2 before the sqrt.)
  - sq-row rank-1 fold and class-pollution fold stacked into ONE
    [33,128]x[33,256] matmul (row 0 = -1/2 x sqrow, rows 1-32 = ct_a).
  - epilogue slots split across DVE / ACT / GpSimd (Pool) per the
    SUM_ENG / CNT_ENG tables; GpSimd also does the f32->bf16 casts.
  - per-core output: (S_p, N_p) pairs; host does the final division.
"""

import numpy as np
from contextlib import ExitStack

import concourse.bass as bass
import concourse.bacc as bacc_mod
import concourse.mybir as mybir
import concourse.tile as tile

F32 = mybir.dt.float32
BF16 = mybir.dt.bfloat16
ALU = mybir.AluOpType
ACTF = mybir.ActivationFunctionType

# problem constants
P_TOT, N, C = 64, 256, 256
K, NCLS = 8, 32
MARGIN = 0.2
NCORES = 8
PPC = P_TOT // NCORES  # parts per core
HALVES = 2  # anchor halves of 128
LBIG = float(2 << 19)  # 2^20 pollution offset
ACC_W = 16  # per-half accumulator cols: 0-7 relu-sums, 8-15 counts

# engine assignment per pos-slot t (per half): 'dve' | 'act'
# (GpSimd/Pool cannot run TensorScalarPtr ops - walrus ISA check)
SUM_ENG = ["dve", "dve", "dve", "dve", "dve", "act", "act", "act"]
CNT_ENG = ["dve", "dve", "dve", "dve", "dve", "dve", "dve", "dve"]


def build_kernel(do_compile=True, reps=1):
    nc = bacc_mod.Bacc()
    x_in = nc.declare_dram_parameter("x", [PPC * N, C], F32, isOutput=False)
    sn_out = nc.declare_dram_parameter("sn", [1, 2 * PPC], F32, isOutput=True)

    with tile.TileContext(nc) as tc, ExitStack() as ctx:
        consts = ctx.enter_context(tc.tile_pool(name="consts", bufs=1))
        xpool = ctx.enter_context(tc.tile_pool(name="xpool", bufs=2))
        xtpool = ctx.enter_context(tc.tile_pool(name="xtpool", bufs=2))
        dpool = ctx.enter_context(tc.tile_pool(name="dpool", bufs=2))
        small = ctx.enter_context(tc.tile_pool(name="small", bufs=3))
        trash = ctx.enter_context(tc.tile_pool(name="trash", bufs=6))
        accp = ctx.enter_context(tc.tile_pool(name="accp", bufs=2))
        psum = ctx.enter_context(tc.tile_pool(name="psum", bufs=2, space="PSUM"))
        psmall = ctx.enter_context(tc.tile_pool(name="psmall", bufs=1, space="PSUM"))

        # ---- one-time constants ----
        # stacked fold lhsT: [33, 256]; row 0 = -1/2 (pairs with sqrow rhs
        # row), rows 1..32 = ct_a = -L/2 * 1[class(col)==row-1]
        lhsT_st = consts.tile([NCLS + 1, N], BF16, tag="lhsT_st")
        nc.vector.memset(lhsT_st[:], -LBIG / 2)
        nc.gpsimd.affine_select(
            lhsT_st[:], lhsT_st[:], pattern=[[1, NCLS], [0, K]],
            compare_op=ALU.is_equal, fill=0.0, base=1, channel_multiplier=-1,
        )
        nc.vector.memset(lhsT_st[0:1, :], -0.5)
        # stacked fold rhs: [33, 256]; row 0 <- sqrow (written per part),
        # rows 1..32 = ct_one = 1[class(col)==row-1]
        rhs_st = consts.tile([NCLS + 1, N], BF16, tag="rhs_st")
        nc.vector.memset(rhs_st[:], 1.0)
        nc.gpsimd.affine_select(
            rhs_st[:], rhs_st[:], pattern=[[1, NCLS], [0, K]],
            compare_op=ALU.is_equal, fill=0.0, base=1, channel_multiplier=-1,
        )
        ident = consts.tile([128, 128], BF16, tag="ident")
        nc.vector.memset(ident[:], 1.0)
        nc.gpsimd.affine_select(
            ident[:], ident[:], pattern=[[1, 128]],
            compare_op=ALU.is_equal, fill=0.0, base=0, channel_multiplier=-1,
        )
        ones_col = consts.tile([128, 1], F32, tag="ones_col")  # final col-sum lhsT
        nc.vector.memset(ones_col[:], 1.0)

        # persistent cross-part accumulator in PSUM
        fin_ps = psmall.tile([1, ACC_W * HALVES * PPC], F32, tag="fin_ps")

        for p in [pp for _ in range(reps) for pp in range(PPC)]:
            # ---- load x rows (f32), cast to bf16 on GpSimd ----
            xf = [xpool.tile([128, C], F32, tag="xf", name="xf", bufs=4) for _ in range(HALVES)]
            for h in range(HALVES):
                nc.sync.dma_start(xf[h][:], x_in[p * N + 128 * h: p * N + 128 * (h + 1), :])
            xb = [xpool.tile([128, C], BF16, tag="xb", name="xb") for _ in range(HALVES)]
            for h in range(HALVES):
                nc.vector.tensor_copy(xb[h][:], xf[h][:])

            # ---- squared norms: sc (f32 col) via DVE STT, scb bf16 ----
            sqcol = []
            for h in range(HALVES):
                sc = small.tile([128, 1], F32, tag="sqcol")
                st = trash.tile([128, C], BF16, tag="trash_sq")
                nc.vector.scalar_tensor_tensor(
                    st[:], xb[h][:], 1.0, xb[h][:],
                    op0=ALU.mult, op1=ALU.mult, accum_out=sc[:],
                )
                sqcol.append(sc)
            sqcol_b = []
            for h in range(HALVES):
                scb = small.tile([128, 1], BF16, tag="sqcolb")
                nc.vector.tensor_copy(scb[:], sqcol[h][:])
                sqcol_b.append(scb)

            # ---- transposes via PE into PSUM ----
            xtps = psum.tile([128, 2 * N], BF16, tag="xtps", name="xtps", bufs=2)
            for cchunk in range(2):
                for h in range(HALVES):
                    nc.tensor.transpose(
                        xtps[:, 256 * cchunk + 128 * h: 256 * cchunk + 128 * (h + 1)],
                        xb[h][:, 128 * cchunk: 128 * (cchunk + 1)],
                        ident[:],
                    )
            sqrow_ps = psum.tile([1, N], BF16, tag="sqrow_ps", name="sqrow_ps", bufs=2)
            for h in range(HALVES):
                nc.tensor.transpose(
                    sqrow_ps[0:1, 128 * h: 128 * (h + 1)], sqcol_b[h][:], ident[:]
                )
            xtb_all = xtpool.tile([128, 2 * N], BF16, tag="xtb", name="xtb")
            nc.vector.tensor_copy(xtb_all[:], xtps[:])
            xtb = [xtb_all[:, 0:N], xtb_all[:, N: 2 * N]]
            # sqrow -> row 0 of the stacked fold rhs (bf16)
            nc.vector.tensor_copy(rhs_st[0:1, :], sqrow_ps[:])

            # ---- both halves' psum: gram + stacked fold ----
            ps = psum.tile([128, 2 * N], F32, tag="ps", bufs=2)
            for h in range(HALVES):
                psh = ps[:, N * h: N * (h + 1)]
                nc.tensor.matmul(
                    psh, xtb[0][:, 128 * h: 128 * (h + 1)], xtb[0][:],
                    start=True, stop=False,
                )
                nc.tensor.matmul(
                    psh, xtb[1][:, 128 * h: 128 * (h + 1)], xtb[1][:],
                    start=False, stop=False,
                )
                nc.tensor.matmul(
                    psh, lhsT_st[:, 128 * h: 128 * (h + 1)], rhs_st[:],
                    start=False, stop=True,
                )

            acc = accp.tile([128, ACC_W * HALVES], F32, tag="acc", name="acc")
            dmat = []
            pm = []
            for h in range(HALVES):
                psh = ps[:, N * h: N * (h + 1)]
                # D' = sqrt(s + L*B): bf16 for DVE fast modes downstream
                dm = dpool.tile([128, N], BF16, tag="dmat")
                nc.scalar.activation(
                    dm[:], psh, ACTF.Sqrt, bias=sqcol[h][:], scale=-2.0,
                )
                dmat.append(dm)

                # pos extraction from the SAME psum: same-class cols of the
                # anchor's half-block sit at s+L -> ps-space minimum.
                spos = small.tile([128, K], F32, tag="spos")
                nc.vector.tensor_reduce(
                    spos[:],
                    psh[:, 128 * h: 128 * (h + 1)].rearrange(
                        "p (cc t) -> p t cc", cc=NCLS // 2, t=K
                    ),
                    axis=mybir.AxisListType.X, op=ALU.min,
                )
                # bias = sqcol - L ; clamp spos <= bias/2 so s_pos >= 0
                sqml = small.tile([128, 1], F32, tag="sqml")
                nc.vector.tensor_scalar(sqml[:], sqcol[h][:], -LBIG, None, op0=ALU.add)
                sqml2 = small.tile([128, 1], F32, tag="sqml2")
                nc.vector.tensor_scalar(sqml2[:], sqml[:], 0.5, None, op0=ALU.mult)
                sposc = small.tile([128, K], F32, tag="sposc")
                nc.vector.tensor_scalar(sposc[:], spos[:], sqml2[:], None, op0=ALU.min)
                pp = small.tile([128, K], F32, tag="pp")
                nc.scalar.activation(pp[:], sposc[:], ACTF.Sqrt, bias=sqml[:], scale=-2.0)
                pmh = small.tile([128, K], F32, tag="pmh")
                nc.vector.tensor_scalar(pmh[:], pp[:], MARGIN, None, op0=ALU.add)
                pm.append(pmh)

            # ---- epilogue: per (half, pos-slot) sum + count ----
            for h in range(HALVES):
                dm = dmat[h]
                base = ACC_W * h
                for t in range(K):
                    s_eng = SUM_ENG[t]
                    a_sum = acc[:, base + t: base + t + 1]
                    if s_eng == "act":
                        t1 = trash.tile([128, N], BF16, tag="trA")
                        nc.scalar.activation(
                            t1[:], dm[:], ACTF.Relu,
                            bias=pm[h][:, t: t + 1], scale=-1.0,
                            accum_out=a_sum,
                        )
                    else:
                        eng = nc.vector if s_eng == "dve" else nc.gpsimd
                        t1 = trash.tile([128, N], BF16, tag="trB")
                        eng.scalar_tensor_tensor(
                            t1[:], dm[:], pm[h][:, t: t + 1], dm[:],
                            op0=ALU.max, op1=ALU.subtract, accum_out=a_sum,
                        )
                    c_eng = CNT_ENG[t]
                    a_cnt = acc[:, base + 8 + t: base + 9 + t]
                    eng = nc.vector if c_eng == "dve" else nc.gpsimd
                    t2 = trash.tile([128, N], BF16, tag="trC")
                    eng.tensor_scalar(
                        t2[:], dm[:], pm[h][:, t: t + 1], None,
                        op0=ALU.is_lt, op1=ALU.add, accum_out=a_cnt,
                    )

            # ---- cross-partition col sums via PE ----
            j = p * ACC_W * HALVES
            nc.tensor.matmul(
                fin_ps[0:1, j: j + ACC_W * HALVES], ones_col[:], acc[:],
                start=True, stop=True,
            )

        # ---- finalize: [1, 16*2*8] -> (S_p, N_p) x 8 ----
        fin = small.tile([1, ACC_W * HALVES * PPC], F32, tag="fin")
        nc.vector.tensor_copy(fin[:], fin_ps[:])
        # relu-sum part: cols 0..7 of each 16-block; counts: cols 8..15
        nblk = PPC * HALVES
        s_ph = small.tile([1, nblk], F32, tag="s_ph")
        nc.vector.tensor_reduce(
            s_ph[:],
            fin[:].rearrange("o (j w) -> o j w", j=nblk, w=ACC_W)[:, :, 0:8],
            axis=mybir.AxisListType.X, op=ALU.add,
        )
        n_ph = small.tile([1, nblk], F32, tag="n_ph")
        nc.vector.tensor_reduce(
            n_ph[:],
            fin[:].rearrange("o (j w) -> o j w", j=nblk, w=ACC_W)[:, :, 8:16],
            axis=mybir.AxisListType.X, op=ALU.add,
        )
        s_p = small.tile([1, PPC], F32, tag="s_p")
        nc.vector.tensor_reduce(
            s_p[:], s_ph[:].rearrange("o (p h) -> o p h", p=PPC, h=HALVES),
            axis=mybir.AxisListType.X, op=ALU.add,
        )
        n_p = small.tile([1, PPC], F32, tag="n_p")
        nc.vector.tensor_reduce(
            n_p[:], n_ph[:].rearrange("o (p h) -> o p h", p=PPC, h=HALVES),
            axis=mybir.AxisListType.X, op=ALU.add,
        )
        both = small.tile([1, 2 * PPC], F32, tag="both")
        nc.vector.tensor_copy(both[:, 0:PPC], s_p[:])
        nc.vector.tensor_copy(both[:, PPC: 2 * PPC], n_p[:])
        nc.sync.dma_start(sn_out[:], both[:])

    if do_compile:
        nc.compile()
    return nc


_NC_CACHE = None


def _get_nc():
    global _NC_CACHE
    if _NC_CACHE is None:
        _NC_CACHE = build_kernel()
    return _NC_CACHE


def kernel(x: np.ndarray, y: np.ndarray) -> np.ndarray:
    from concourse.bass_utils import run_bass_kernel_spmd

    x = np.asarray(x)
    y = np.asarray(y)
    assert x.shape == (P_TOT, N, C) and y.shape == (P_TOT, N)
    # the kernel hardcodes the class structure; verify it holds
    expect = np.repeat(np.arange(NCLS, dtype=np.int64), K)
    assert np.array_equal(y, np.broadcast_to(expect, (P_TOT, N))), (
        "kernel requires y[p, i] == i // 8"
    )
    nc = _get_nc()
    xs = np.ascontiguousarray(x.reshape(NCORES, PPC * N, C).astype(np.float32))
    in_maps = [{"x": xs[i]} for i in range(NCORES)]
    res = run_bass_kernel_spmd(nc, in_maps, list(range(NCORES)))
    out = np.empty((P_TOT,), np.float32)
    for i in range(NCORES):
        sn = res.results[i]["sn"].reshape(2 * PPC)
        s, n = sn[:PPC], sn[PPC:]
        out[i * PPC: (i + 1) * PPC] = np.where(n <= 0, 0.0, s / np.maximum(n, 1.0))
    return out
